# revision 18
# baseline (speedup 1.0000x reference)
"""ExponentialStateMixer Trainium2 kernel.

Reference computation (per batch b, source s, channel d):
    h = x @ W_in + b_in                  # [T, 2D]
    alpha = sigmoid(h[:, :D]); cand = tanh(h[:, D:])
    state_t = (1-alpha_t) * state_{t-1} + alpha_t * cand_t   (state_-1 = 0)
    out = state @ W_out + b_out          # [T, D]

Sharding: data-parallel over batch B=8 across the 8 NeuronCores (the time
recurrence is independent per (b, s, d)); projection weights replicated.

Per-core dataflow (per 512-timestep slab):
  - HWDGE load x slab fp32 (contiguous 8 KB runs)
  - DVE cast fp32 -> bf16
  - PE transpose [t, d] -> [d, t] via identity matmul (the scan runs along
    the free dim), PSUM -> SBUF copy
  - PE in_proj (bf16): hT[e, t] = W_in^T xT, PSUM
  - ACT: a = sigmoid(-(gate + b_in_g)) == 1-alpha ; tc = tanh(cand + b_in_c)
  - DVE scalar_tensor_tensor: bneg = (a - 1) * tc == -alpha*tanh(cand)
  - DVE tensor_tensor_scan: yneg_t = a_t * yneg_{t-1} + bneg_t == -state_t
    (chained across slabs through tiny per-(s,d-half) carry tiles)
  - PE out_proj with yneg chunks stationary against (-W_out) -> out[t, e]
    directly in the HBM layout; b_out added via a K=1 matmul when nonzero.
  - PSUM -> SBUF copy, strided DMA store.
"""

import numpy as np
import ml_dtypes

import concourse.bacc as bacc
import concourse.mybir as mybir
import concourse.tile as tile
from concourse import masks
from concourse.bass_utils import run_bass_kernel_spmd

B, T, S, D = 8, 2048, 8, 256
N_CORES = 8
SLAB = 512          # timesteps per pipeline slab
P = 128             # partitions
F32 = mybir.dt.float32
BF16 = mybir.dt.bfloat16
AF = mybir.ActivationFunctionType
ALU = mybir.AluOpType


def build_program(t_len=T, n_src=S, n_cores=N_CORES, repeat=1, with_bout=True,
                  with_bin=True):
    """Build the per-core (SPMD) Bass program. Each core processes one batch
    element: x shard [t_len, n_src, D] -> out [t_len, n_src, D].

    repeat>1 re-runs the whole pipeline (same inputs/outputs) for marginal
    wall-clock timing of the device portion. with_bout=False elides the
    K=1 bias matmuls (used when b_out is all zeros); with_bin=False merges
    the gate/cand e-halves into single wide PSUM tiles and activation ops
    (valid when b_in is all zeros)."""
    assert t_len % SLAB == 0 and D == 2 * P
    n_slab = t_len // SLAB
    n_tsub = SLAB // P   # 128-row blocks per slab

    nc = bacc.Bacc("TRN2", target_bir_lowering=False, debug=False,
                   num_devices=n_cores)

    x_d = nc.dram_tensor("x", [t_len, n_src, D], F32, kind="ExternalInput")
    w_in_d = nc.dram_tensor("w_in_bf16", [D, 2 * D], BF16, kind="ExternalInput")
    w_out_d = nc.dram_tensor("w_out_neg_bf16", [D, D], BF16, kind="ExternalInput")
    bg_d = nc.dram_tensor("b_gate_neg", [D, 1], F32, kind="ExternalInput")
    bc_d = nc.dram_tensor("b_cand", [D, 1], F32, kind="ExternalInput")
    bo_d = nc.dram_tensor("b_out_bf16", [1, D], BF16, kind="ExternalInput")
    out_d = nc.dram_tensor("out", [t_len, n_src, D], F32, kind="ExternalOutput")

    x_ap = x_d.ap()
    out_ap = out_d.ap()

    with tile.TileContext(nc) as tc:
        with (
            tc.tile_pool(name="const", bufs=1) as const_pool,
            tc.tile_pool(name="xcf", bufs=2) as xcf_pool,
            tc.tile_pool(name="xcb", bufs=2) as xcb_pool,
            tc.tile_pool(name="xt", bufs=6) as xt_pool,
            tc.tile_pool(name="act", bufs=6) as act_pool,
            tc.tile_pool(name="y", bufs=10) as y_pool,
            tc.tile_pool(name="ot", bufs=2) as out_pool,
            tc.tile_pool(name="pt", bufs=2, space="PSUM") as pt_pool,
            tc.tile_pool(name="ph", bufs=(4 if with_bin else 2),
                         space="PSUM") as ph_pool,
            tc.tile_pool(name="po", bufs=2, space="PSUM") as po_pool,
        ):
            # ---- constants (once per core) ----
            ident = const_pool.tile([P, P], BF16, tag="ident")
            masks.make_identity(nc, ident[:])
            w_in_sb = []      # per d-half: [128 d, 512 e] bf16
            w_out_sb = []     # per d-half: [128 d, 256 e] bf16 (negated)
            bias_g = []       # per gate e-half: [128, 1] f32 (= -b_in)
            bias_c = []       # per cand e-half: [128, 1] f32
            for dh in range(2):
                w = const_pool.tile([P, 2 * D], BF16, tag=f"w_in{dh}")
                nc.sync.dma_start(w[:], w_in_d.ap()[dh * P:(dh + 1) * P, :])
                w_in_sb.append(w)
                wo = const_pool.tile([P, D], BF16, tag=f"w_out{dh}")
                nc.sync.dma_start(wo[:], w_out_d.ap()[dh * P:(dh + 1) * P, :])
                w_out_sb.append(wo)
                bg = const_pool.tile([P, 1], F32, tag=f"bg{dh}")
                nc.sync.dma_start(bg[:], bg_d.ap()[dh * P:(dh + 1) * P, :])
                bias_g.append(bg)
                bc = const_pool.tile([P, 1], F32, tag=f"bc{dh}")
                nc.sync.dma_start(bc[:], bc_d.ap()[dh * P:(dh + 1) * P, :])
                bias_c.append(bc)
            b_out_sb = const_pool.tile([1, D], BF16, tag="b_out")
            nc.sync.dma_start(b_out_sb[:], bo_d.ap()[:, :])
            ones_sb = const_pool.tile([1, P], BF16, tag="ones")
            nc.gpsimd.memset(ones_sb[:], 1.0)

            y_prev = {}
            for _rep in range(repeat):
                for sl in range(n_slab):
                    # ---- load slab fp32 in two halves, cast to bf16 ----
                    xcb = xcb_pool.tile([P, n_tsub * n_src * D], BF16, tag="xcb")
                    for half in range(2):
                        hw = n_tsub // 2 * n_src * D      # free elems per half
                        xcf = xcf_pool.tile([P, hw], F32, tag="xcf", name="xcf")
                        t0 = sl * SLAB + half * (SLAB // 2)
                        src = x_ap[t0:t0 + SLAB // 2, :, :].rearrange(
                            "(n p) s d -> p n s d", p=P)
                        dst = xcf[:].rearrange("p (n s d) -> p n s d",
                                               n=n_tsub // 2, s=n_src)
                        eng = nc.sync if half == 0 else nc.scalar
                        eng.dma_start(dst, src)
                        nc.vector.tensor_copy(xcb[:, half * hw:(half + 1) * hw],
                                              xcf[:])

                    ot = out_pool.tile([P, n_tsub * n_src * D], F32,
                                       tag="ot", name="ot")
                    for s in range(n_src):
                        # ---- PE transpose [t, d] -> [d, t] ----
                        xt = [xt_pool.tile([P, SLAB], BF16, tag=f"xt{dh}",
                                           name=f"xt{dh}") for dh in range(2)]
                        for dh in range(2):
                            pt = pt_pool.tile([P, SLAB], BF16, tag="pt",
                                              name="pt")
                            for ts_ in range(n_tsub):
                                blk = xcb[:, (ts_ * n_src + s) * D + dh * P:
                                          (ts_ * n_src + s) * D + dh * P + P]
                                nc.tensor.transpose(
                                    pt[:, ts_ * P:(ts_ + 1) * P], blk, ident[:])
                            nc.vector.tensor_copy(xt[dh][:], pt[:])

                        # ---- in_proj + activations, gate then cand ----
                        if with_bin:
                            a_t = [act_pool.tile([P, SLAB], BF16, tag=f"a{dh}",
                                                 name=f"a{dh}")
                                   for dh in range(2)]
                            c_t = [act_pool.tile([P, SLAB], BF16, tag=f"c{dh}",
                                                 name=f"c{dh}")
                                   for dh in range(2)]
                            for grp in range(2):          # 0: gate, 1: cand
                                ph = [ph_pool.tile([P, SLAB], F32, tag="ph",
                                                   name="ph")
                                      for _ in range(2)]
                                for eq in range(2):
                                    for dh in range(2):
                                        nc.tensor.matmul(
                                            ph[eq][:],
                                            w_in_sb[dh][:, (grp * 2 + eq) * P:
                                                        (grp * 2 + eq + 1) * P],
                                            xt[dh][:],
                                            start=(dh == 0), stop=(dh == 1))
                                for eq in range(2):
                                    if grp == 0:
                                        nc.scalar.activation(
                                            a_t[eq][:], ph[eq][:], AF.Sigmoid,
                                            bias=bias_g[eq][:], scale=-1.0)
                                    else:
                                        nc.scalar.activation(
                                            c_t[eq][:], ph[eq][:], AF.Tanh,
                                            bias=bias_c[eq][:], scale=1.0)
                            a_sl = [a_t[0][:], a_t[1][:]]
                            c_sl = [c_t[0][:], c_t[1][:]]
                            stt_ops = [(a_t[0][:], c_t[0][:]),
                                       (a_t[1][:], c_t[1][:])]
                        else:
                            # b_in == 0: one wide PSUM tile + one act per grp
                            a_c = act_pool.tile([P, 2 * SLAB], BF16, tag="ac",
                                                name="ac")
                            c_c = act_pool.tile([P, 2 * SLAB], BF16, tag="cc",
                                                name="cc")
                            for grp in range(2):          # 0: gate, 1: cand
                                ph = ph_pool.tile([P, 2 * SLAB], F32, tag="ph",
                                                  name="ph")
                                for eq in range(2):
                                    for dh in range(2):
                                        nc.tensor.matmul(
                                            ph[:, eq * SLAB:(eq + 1) * SLAB],
                                            w_in_sb[dh][:, (grp * 2 + eq) * P:
                                                        (grp * 2 + eq + 1) * P],
                                            xt[dh][:],
                                            start=(dh == 0), stop=(dh == 1))
                                if grp == 0:
                                    nc.scalar.activation(a_c[:], ph[:],
                                                         AF.Sigmoid,
                                                         scale=-1.0)
                                else:
                                    nc.scalar.activation(c_c[:], ph[:],
                                                         AF.Tanh)
                            a_sl = [a_c[:, 0:SLAB], a_c[:, SLAB:2 * SLAB]]
                            c_sl = [c_c[:, 0:SLAB], c_c[:, SLAB:2 * SLAB]]
                            stt_ops = [(a_c[:], c_c[:])]

                        # ---- bneg = (a-1)*tc via 4x ts + 2x tt ----
                        for a_ap, c_ap in stt_ops:
                            am1 = act_pool.tile([P, a_ap.shape[1]], BF16,
                                                tag="am1", name="am1")
                            nc.vector.tensor_scalar(am1[:], a_ap, -1.0, None,
                                                    ALU.add)
                            nc.vector.tensor_mul(c_ap, am1[:], c_ap)
                        y_sl = [None, None]
                        for dh in range(2):
                            y = y_pool.tile([P, SLAB], BF16, tag=f"y{dh}",
                                            name=f"y{dh}")
                            init = (0.0 if sl == 0 else
                                    y_prev[(s, dh)][:, SLAB - 1:SLAB])
                            nc.vector.tensor_tensor_scan(
                                y[:], a_sl[dh], c_sl[dh], init,
                                ALU.mult, ALU.add)
                            y_prev[(s, dh)] = y
                            y_sl[dh] = y

                        # ---- out_proj -> out[t, e] in PSUM ----
                        otv = ot[:].rearrange("p (n s d) -> p n s d",
                                              n=n_tsub, s=n_src)
                        for half in range(n_tsub // 2):
                            po = po_pool.tile([P, 2 * D], F32, tag="po",
                                              name="po")
                            for k in range(2):
                                ts_ = half * 2 + k
                                dst = po[:, k * D:(k + 1) * D]
                                if with_bout:
                                    nc.tensor.matmul(dst, ones_sb[:],
                                                     b_out_sb[:],
                                                     start=True, stop=False)
                                for dh in range(2):
                                    nc.tensor.matmul(
                                        dst,
                                        y_sl[dh][:, ts_ * P:(ts_ + 1) * P],
                                        w_out_sb[dh][:],
                                        start=(not with_bout and dh == 0),
                                        stop=(dh == 1))
                            nc.scalar.activation(
                                otv[:, half * 2:half * 2 + 2, s, :],
                                po[:].rearrange("p (k d) -> p k d", k=2),
                                AF.Copy)

                    # ---- store whole slab: out[sl*512 + ..., :, :] ----
                    dst = out_ap[sl * SLAB:(sl + 1) * SLAB, :, :].rearrange(
                        "(n p) s d -> p n s d", p=P)
                    osrc = ot[:].rearrange("p (n s d) -> p n s d",
                                           n=n_tsub, s=n_src)
                    seng = nc.scalar if sl % 2 == 0 else nc.sync
                    seng.dma_start(dst, osrc)

    nc.compile()
    return nc


_CACHE = {}


def _get_program(repeat=1, with_bout=True, with_bin=True):
    key = ("nc", repeat, with_bout, with_bin)
    if key not in _CACHE:
        _CACHE[key] = build_program(repeat=repeat, with_bout=with_bout,
                                    with_bin=with_bin)
    return _CACHE[key]


def _make_in_maps(x, W_in, b_in, W_out, b_out, n_cores=N_CORES):
    w_in_bf16 = np.ascontiguousarray(W_in.astype(ml_dtypes.bfloat16))
    w_out_neg = np.ascontiguousarray((-W_out).astype(ml_dtypes.bfloat16))
    b_gate_neg = np.ascontiguousarray(-b_in[:D].astype(np.float32))[:, None]
    b_cand = np.ascontiguousarray(b_in[D:].astype(np.float32))[:, None]
    b_out_bf16 = np.ascontiguousarray(b_out.astype(ml_dtypes.bfloat16))[None, :]
    return [
        {
            "x": np.ascontiguousarray(x[b]),
            "w_in_bf16": w_in_bf16,
            "w_out_neg_bf16": w_out_neg,
            "b_gate_neg": b_gate_neg,
            "b_cand": b_cand,
            "b_out_bf16": b_out_bf16,
        }
        for b in range(n_cores)
    ]


def run(x, W_in, b_in, W_out, b_out, trace=False, repeat=1, **spmd_kwargs):
    with_bout = bool(np.any(b_out))
    with_bin = bool(np.any(b_in))
    nc = _get_program(repeat=repeat, with_bout=with_bout, with_bin=with_bin)
    in_maps = _make_in_maps(x, W_in, b_in, W_out, b_out)
    res = run_bass_kernel_spmd(nc, in_maps, list(range(N_CORES)),
                               trace=trace, **spmd_kwargs)
    out = np.stack([res.results[b]["out"] for b in range(N_CORES)], axis=0)
    return out.astype(np.float32), res


def kernel(x, W_in, b_in, W_out, b_out):
    out, _ = run(np.asarray(x), np.asarray(W_in), np.asarray(b_in),
                 np.asarray(W_out), np.asarray(b_out))
    return out
